# revision 3
# baseline (speedup 1.0000x reference)
"""Conv2d 3x3 (stride 1, pad 1) as implicit GEMM on 8 Trainium2 NeuronCores.

Problem: x [32,128,56,56] f32, weight [256,128,3,3] f32, bias [256] f32
         -> out [32,256,56,56] f32.

Sharding: data-parallel over batch. Each of the 8 cores gets 4 images;
weight/bias are replicated. No collectives; outputs are concatenated on host.

Per-core kernel (implicit GEMM):
  - x is host-padded to [4,128,58,58]; one image's padded plane lives in SBUF
    as a [128, 58, 58] tile (in-channels on partitions).
  - weight is host-rearranged to [128, 9, 256] (in-ch partitions, 3x3 taps,
    out-ch free) so lhsT slices are direct.
  - For each image, out-channel group g (2 groups of 128) and band of 8
    output rows (7 bands): accumulate 9 matmuls (one per tap) into a
    [128, 448] PSUM tile:  psum += W[:, ki, g*128:...].T @ xpad[:, rows+kh, kw:kw+56]
  - bias-add + PSUM->SBUF copy on the scalar engine, then DMA to DRAM.
"""

import numpy as np

import concourse.bacc as bacc
import concourse.mybir as mybir
import concourse.tile as tile
from concourse.bass_utils import run_bass_kernel_spmd

N_CORES = 8
B, C_IN, H, W = 32, 128, 56, 56
C_OUT = 256
KH = KW = 3
B_LOC = B // N_CORES          # 4 images per core
HP, WP = H + 2, W + 2         # 58 (pad=1)
ROWS = 8                      # output rows per matmul
NCHUNK = H // ROWS            # 7 bands
NFREE = ROWS * W              # 448 = matmul free dim (fits one PSUM bank)
NGRP = C_OUT // 128           # 2 out-channel groups

# float32r is the full-rate fp32 PE mode (1 cycle/row for N>=256 vs 4 for
# plain fp32). Flip to mybir.dt.float32 if hardware numerics disappoint.
MM_DT = mybir.dt.float32


def _build():
    nc = bacc.Bacc(None, target_bir_lowering=False)
    xp = nc.dram_tensor(
        "xp", [B_LOC, C_IN, HP, WP], mybir.dt.float32, kind="ExternalInput"
    )
    wt = nc.dram_tensor(
        "wt", [C_IN, KH * KW, C_OUT], mybir.dt.float32, kind="ExternalInput"
    )
    bz = nc.dram_tensor("bz", [128, NGRP], mybir.dt.float32, kind="ExternalInput")
    out = nc.dram_tensor(
        "out", [B_LOC, NGRP, 128, H * W], mybir.dt.float32, kind="ExternalOutput"
    )

    with tile.TileContext(nc) as tc:
        with (
            tc.tile_pool(name="const", bufs=1) as cpool,
            tc.tile_pool(name="xin", bufs=2) as xpool,
            tc.tile_pool(name="oout", bufs=4) as opool,
            tc.tile_pool(name="psum", bufs=4, space="PSUM") as pspool,
        ):
            w_tile = cpool.tile([C_IN, KH * KW, C_OUT], mybir.dt.float32)
            nc.sync.dma_start(w_tile[:], wt[:])
            b_tile = cpool.tile([128, NGRP], mybir.dt.float32)
            nc.sync.dma_start(b_tile[:], bz[:])

            for b in range(B_LOC):
                x_tile = xpool.tile([C_IN, HP, WP], mybir.dt.float32)
                nc.sync.dma_start(x_tile[:], xp[b])
                for g in range(NGRP):
                    for rc in range(NCHUNK):
                        ps = pspool.tile([128, NFREE], mybir.dt.float32)
                        for kh in range(KH):
                            for kw in range(KW):
                                ki = kh * KW + kw
                                lhsT = w_tile[:, ki, g * 128 : (g + 1) * 128]
                                rhs = x_tile[
                                    :,
                                    rc * ROWS + kh : rc * ROWS + kh + ROWS,
                                    kw : kw + W,
                                ]
                                if MM_DT != mybir.dt.float32:
                                    lhsT = lhsT.bitcast(MM_DT)
                                    rhs = rhs.bitcast(MM_DT)
                                nc.tensor.matmul(
                                    ps[:],
                                    lhsT,
                                    rhs,
                                    start=(ki == 0),
                                    stop=(ki == KH * KW - 1),
                                )
                        o_tile = opool.tile([128, NFREE], mybir.dt.float32)
                        nc.scalar.activation(
                            o_tile[:],
                            ps[:],
                            mybir.ActivationFunctionType.Identity,
                            bias=b_tile[:, g : g + 1],
                            scale=1.0,
                        )
                        nc.sync.dma_start(
                            out[b, g, :, rc * NFREE : (rc + 1) * NFREE], o_tile[:]
                        )
    nc.finalize()
    return nc


_NC = None


def _prep_inputs(x, weight, bias):
    x = np.asarray(x, dtype=np.float32)
    weight = np.asarray(weight, dtype=np.float32)
    bias = np.asarray(bias, dtype=np.float32)
    xp = np.zeros((B, C_IN, HP, WP), dtype=np.float32)
    xp[:, :, 1 : H + 1, 1 : W + 1] = x
    # wt[p, kh*3+kw, o] = weight[o, p, kh, kw]
    wt = np.ascontiguousarray(
        weight.transpose(1, 2, 3, 0).reshape(C_IN, KH * KW, C_OUT)
    )
    # bz[p, g] = bias[g*128 + p]
    bz = np.ascontiguousarray(bias.reshape(NGRP, 128).T)
    return xp, wt, bz


def kernel(x, weight, bias, trace=False):
    global _NC
    xp, wt, bz = _prep_inputs(x, weight, bias)
    if _NC is None:
        _NC = _build()
    in_maps = [
        {"xp": xp[c * B_LOC : (c + 1) * B_LOC], "wt": wt, "bz": bz}
        for c in range(N_CORES)
    ]
    res = run_bass_kernel_spmd(
        _NC, in_maps, core_ids=list(range(N_CORES)), trace=trace
    )
    outs = [r["out"].reshape(B_LOC, C_OUT, H, W) for r in res.results]
    full = np.concatenate(outs, axis=0)
    if trace:
        return full, res
    return full


# revision 8
# speedup vs baseline: 3.0533x; 3.0533x over previous
"""Conv2d 3x3 (stride 1, pad 1) as implicit GEMM on 8 Trainium2 NeuronCores.

Problem: x [32,128,56,56] f32, weight [256,128,3,3] f32, bias [256] f32
         -> out [32,256,56,56] f32.

Sharding: data-parallel over batch. Each of the 8 cores gets 4 images;
weight/bias are replicated. No collectives; outputs are concatenated on host.

Per-core kernel (implicit GEMM):
  - x is host-padded to [4,128,58,58]; one image's padded plane lives in SBUF
    as a [128, 58, 58] tile (in-channels on partitions).
  - weight is host-rearranged to [128, 9, 256] (in-ch partitions, 3x3 taps,
    out-ch free) so lhsT slices are direct.
  - For each image, out-channel group g (2 groups of 128) and band of 8
    output rows (7 bands): accumulate 9 matmuls (one per tap) into a
    [128, 448] PSUM tile:  psum += W[:, ki, g*128:...].T @ xpad[:, rows+kh, kw:kw+56]
  - bias-add + PSUM->SBUF copy on the scalar engine, then DMA to DRAM.
"""

import numpy as np

import concourse.bacc as bacc
import concourse.mybir as mybir
import concourse.tile as tile
from concourse.bass_utils import run_bass_kernel_spmd

N_CORES = 8
B, C_IN, H, W = 32, 128, 56, 56
C_OUT = 256
KH = KW = 3
B_LOC = B // N_CORES          # 4 images per core
HP, WP = H + 2, W + 2         # 58 (pad=1)
ROWS = 8                      # output rows per matmul
NCHUNK = H // ROWS            # 7 bands
NFREE = ROWS * W              # 448 = matmul free dim (fits one PSUM bank)
NGRP = C_OUT // 128           # 2 out-channel groups

# float32r is the full-rate fp32 PE mode (1 cycle/row for N>=256 vs 4 for
# plain fp32). Flip to mybir.dt.float32 if hardware numerics disappoint.
MM_DT = mybir.dt.float32r


def _build():
    nc = bacc.Bacc(None, target_bir_lowering=False)
    xp = nc.dram_tensor("xp", [B_LOC, C_IN, HP, WP], MM_DT, kind="ExternalInput")
    wt = nc.dram_tensor("wt", [C_IN, KH * KW, C_OUT], MM_DT, kind="ExternalInput")
    bz = nc.dram_tensor("bz", [128, NGRP], mybir.dt.float32, kind="ExternalInput")
    out = nc.dram_tensor(
        "out", [B_LOC, NGRP, 128, H * W], mybir.dt.float32, kind="ExternalOutput"
    )

    with tile.TileContext(nc) as tc:
        with (
            tc.tile_pool(name="const", bufs=1) as cpool,
            tc.tile_pool(name="xin", bufs=2) as xpool,
            tc.tile_pool(name="oout", bufs=4) as opool,
            tc.tile_pool(name="psum", bufs=4, space="PSUM") as pspool,
        ):
            w_tile = cpool.tile([C_IN, KH * KW, C_OUT], MM_DT)
            nc.sync.dma_start(w_tile[:], wt[:])
            b_tile = cpool.tile([128, NGRP], mybir.dt.float32)
            nc.sync.dma_start(b_tile[:], bz[:])

            for b in range(B_LOC):
                x_tile = xpool.tile([C_IN, HP, WP], MM_DT)
                nc.sync.dma_start(x_tile[:], xp[b])
                for g in range(NGRP):
                    for rc in range(NCHUNK):
                        ps = pspool.tile([128, NFREE], mybir.dt.float32)
                        for kh in range(KH):
                            for kw in range(KW):
                                ki = kh * KW + kw
                                lhsT = w_tile[:, ki, g * 128 : (g + 1) * 128]
                                rhs = x_tile[
                                    :,
                                    rc * ROWS + kh : rc * ROWS + kh + ROWS,
                                    kw : kw + W,
                                ]
                                nc.tensor.matmul(
                                    ps[:],
                                    lhsT,
                                    rhs,
                                    start=(ki == 0),
                                    stop=(ki == KH * KW - 1),
                                )
                        o_tile = opool.tile([128, NFREE], mybir.dt.float32)
                        nc.scalar.activation(
                            o_tile[:],
                            ps[:],
                            mybir.ActivationFunctionType.Identity,
                            bias=b_tile[:, g : g + 1],
                            scale=1.0,
                        )
                        nc.sync.dma_start(
                            out[b, g, :, rc * NFREE : (rc + 1) * NFREE], o_tile[:]
                        )
    nc.finalize()
    return nc


_NC = None


def _prep_inputs(x, weight, bias):
    x = np.asarray(x, dtype=np.float32)
    weight = np.asarray(weight, dtype=np.float32)
    bias = np.asarray(bias, dtype=np.float32)
    xp = np.zeros((B, C_IN, HP, WP), dtype=np.float32)
    xp[:, :, 1 : H + 1, 1 : W + 1] = x
    # wt[p, kh*3+kw, o] = weight[o, p, kh, kw]
    wt = np.ascontiguousarray(
        weight.transpose(1, 2, 3, 0).reshape(C_IN, KH * KW, C_OUT)
    )
    # bz[p, g] = bias[g*128 + p]
    bz = np.ascontiguousarray(bias.reshape(NGRP, 128).T)
    return xp, wt, bz


def kernel(x, weight, bias, trace=False):
    global _NC
    xp, wt, bz = _prep_inputs(x, weight, bias)
    if _NC is None:
        _NC = _build()
    in_maps = [
        {"xp": xp[c * B_LOC : (c + 1) * B_LOC], "wt": wt, "bz": bz}
        for c in range(N_CORES)
    ]
    res = run_bass_kernel_spmd(
        _NC, in_maps, core_ids=list(range(N_CORES)), trace=trace
    )
    outs = [r["out"].reshape(B_LOC, C_OUT, H, W) for r in res.results]
    full = np.concatenate(outs, axis=0)
    if trace:
        return full, res
    return full


# revision 9
# speedup vs baseline: 3.0955x; 1.0138x over previous
"""Conv2d 3x3 (stride 1, pad 1) as implicit GEMM on 8 Trainium2 NeuronCores.

Problem: x [32,128,56,56] f32, weight [256,128,3,3] f32, bias [256] f32
         -> out [32,256,56,56] f32.

Sharding: data-parallel over batch. Each of the 8 cores gets 4 images;
weight/bias are replicated. No collectives; outputs are concatenated on host.

Per-core kernel (implicit GEMM):
  - x is host-padded to [4,128,58,58]; one image's padded plane lives in SBUF
    as a [128, 58, 58] tile (in-channels on partitions).
  - weight is host-rearranged to [128, 9, 256] (in-ch partitions, 3x3 taps,
    out-ch free) so lhsT slices are direct.
  - For each image, out-channel group g (2 groups of 128) and band of 8
    output rows (7 bands): accumulate 9 matmuls (one per tap) into a
    [128, 448] PSUM tile:  psum += W[:, ki, g*128:...].T @ xpad[:, rows+kh, kw:kw+56]
  - bias-add + PSUM->SBUF copy on the scalar engine, then DMA to DRAM.
"""

import numpy as np

import concourse.bacc as bacc
import concourse.mybir as mybir
import concourse.tile as tile
from concourse.bass_utils import run_bass_kernel_spmd

N_CORES = 8
B, C_IN, H, W = 32, 128, 56, 56
C_OUT = 256
KH = KW = 3
B_LOC = B // N_CORES          # 4 images per core
HP, WP = H + 2, W + 2         # 58 (pad=1)
ROWS = 8                      # output rows per matmul
NCHUNK = H // ROWS            # 7 bands
NFREE = ROWS * W              # 448 = matmul free dim (fits one PSUM bank)
NGRP = C_OUT // 128           # 2 out-channel groups

# float32r is the full-rate fp32 PE mode (1 cycle/row for N>=256 vs 4 for
# plain fp32). Flip to mybir.dt.float32 if hardware numerics disappoint.
MM_DT = mybir.dt.float32r


def _build():
    nc = bacc.Bacc(None, target_bir_lowering=False)
    xp = nc.dram_tensor("xp", [B_LOC, C_IN, HP, WP], MM_DT, kind="ExternalInput")
    wt = nc.dram_tensor("wt", [C_IN, KH * KW, C_OUT], MM_DT, kind="ExternalInput")
    bz = nc.dram_tensor("bz", [128, NGRP], mybir.dt.float32, kind="ExternalInput")
    out = nc.dram_tensor(
        "out", [B_LOC, NGRP, 128, H * W], mybir.dt.float32, kind="ExternalOutput"
    )

    with tile.TileContext(nc) as tc:
        with (
            tc.tile_pool(name="const", bufs=1) as cpool,
            tc.tile_pool(name="xin", bufs=2) as xpool,
            tc.tile_pool(name="oout", bufs=4) as opool,
            tc.tile_pool(name="psum", bufs=4, space="PSUM") as pspool,
        ):
            w_tile = cpool.tile([C_IN, KH * KW, C_OUT], MM_DT)
            # split across taps so the load spreads over DMA queues
            for ki in range(KH * KW):
                nc.sync.dma_start(w_tile[:, ki], wt[:, ki])
            b_tile = cpool.tile([128, NGRP], mybir.dt.float32)
            nc.sync.dma_start(b_tile[:], bz[:])

            for b in range(B_LOC):
                x_tile = xpool.tile([C_IN, HP, WP], MM_DT)
                # band-aligned row chunks: band rc needs padded rows
                # [rc*ROWS, rc*ROWS+ROWS+2); chunk 0 covers rows 0..9,
                # chunk rc>=1 adds rows rc*ROWS+2 .. rc*ROWS+9.
                row_hi = 0
                for rc in range(NCHUNK):
                    row_lo = row_hi
                    row_hi = rc * ROWS + ROWS + 2
                    nc.sync.dma_start(
                        x_tile[:, row_lo:row_hi], xp[b, :, row_lo:row_hi]
                    )
                for g in range(NGRP):
                    for rc in range(NCHUNK):
                        ps = pspool.tile([128, NFREE], mybir.dt.float32)
                        for kh in range(KH):
                            for kw in range(KW):
                                ki = kh * KW + kw
                                lhsT = w_tile[:, ki, g * 128 : (g + 1) * 128]
                                rhs = x_tile[
                                    :,
                                    rc * ROWS + kh : rc * ROWS + kh + ROWS,
                                    kw : kw + W,
                                ]
                                nc.tensor.matmul(
                                    ps[:],
                                    lhsT,
                                    rhs,
                                    start=(ki == 0),
                                    stop=(ki == KH * KW - 1),
                                )
                        o_tile = opool.tile([128, NFREE], mybir.dt.float32)
                        nc.scalar.activation(
                            o_tile[:],
                            ps[:],
                            mybir.ActivationFunctionType.Identity,
                            bias=b_tile[:, g : g + 1],
                            scale=1.0,
                        )
                        nc.sync.dma_start(
                            out[b, g, :, rc * NFREE : (rc + 1) * NFREE], o_tile[:]
                        )
    nc.finalize()
    return nc


_NC = None


def _prep_inputs(x, weight, bias):
    x = np.asarray(x, dtype=np.float32)
    weight = np.asarray(weight, dtype=np.float32)
    bias = np.asarray(bias, dtype=np.float32)
    xp = np.zeros((B, C_IN, HP, WP), dtype=np.float32)
    xp[:, :, 1 : H + 1, 1 : W + 1] = x
    # wt[p, kh*3+kw, o] = weight[o, p, kh, kw]
    wt = np.ascontiguousarray(
        weight.transpose(1, 2, 3, 0).reshape(C_IN, KH * KW, C_OUT)
    )
    # bz[p, g] = bias[g*128 + p]
    bz = np.ascontiguousarray(bias.reshape(NGRP, 128).T)
    return xp, wt, bz


def kernel(x, weight, bias, trace=False):
    global _NC
    xp, wt, bz = _prep_inputs(x, weight, bias)
    if _NC is None:
        _NC = _build()
    in_maps = [
        {"xp": xp[c * B_LOC : (c + 1) * B_LOC], "wt": wt, "bz": bz}
        for c in range(N_CORES)
    ]
    res = run_bass_kernel_spmd(
        _NC, in_maps, core_ids=list(range(N_CORES)), trace=trace
    )
    outs = [r["out"].reshape(B_LOC, C_OUT, H, W) for r in res.results]
    full = np.concatenate(outs, axis=0)
    if trace:
        return full, res
    return full
